# revision 7
# baseline (speedup 1.0000x reference)
"""Trainium2 Bass kernel for nn_ClassNetPP (segment_reduce).

Data-parallel over batch: 32 samples -> 8 NeuronCores x 4 samples.
Per-core pipeline (channels-on-partitions layout [C, N] with N=32x32 spatial):
  A: 1x1 conv w_in (BN1 folded) + ReLU          -> y1 [512, 1024]
  B: 1x1 conv w_reduce + ReLU into padded 36x36 -> zp [128, 1444]
  C: 3x3 convs (dil 1,2) as 9 shifted matmuls   -> zl, zg [128, 1024]
  D: 1x1 conv w_rcb_out (BN2 folded) + residual + ReLU -> y2 [512, 1024]
  E: x3 = w_out @ y2 computed as y2-slices-as-stationary -> [N, D] layout
  F: sims = (protos @ w_out) @ y2 (host-fused Q), scaled by 4/||x3||,
     exp on ACT, per-class sum over K=32 -> ksum, transposed to [C, N]
Host post-processing does the two logs (act_maps, logits) from ksum sums.
"""

import numpy as np

import concourse.bacc as bacc
import concourse.bass as bass
import concourse.mybir as mybir
import concourse.tile as tile
from concourse.bass_utils import run_bass_kernel_spmd

F32 = mybir.dt.float32
F32R = mybir.dt.float32r
I32 = mybir.dt.int32
AF = mybir.ActivationFunctionType
ALU = mybir.AluOpType
AX = mybir.AxisListType

N_CORES = 8
SPC = 4            # samples per core
B, N, DIN, D, HD, C, K = 32, 1024, 768, 512, 128, 20, 32
S = 32             # spatial side
W = 36             # padded spatial side (pad 2 each side for dilation 2)
MARGIN = 74        # 2*36+2: max |tap offset| in padded-flat space
TAU = 0.25
BN_EPS = 1e-5

TRACE = False      # set kernel.TRACE = True before calling kernel() to profile
LAST_RESULT = None # BassKernelResults of the last run (for profiling info)


def _r(ap):
    return ap


def build_bass(spc=SPC):
    nc = bacc.Bacc()

    tokt_d = nc.dram_tensor("tokt", [spc, DIN, N], F32R, kind="ExternalInput")
    w1_d = nc.dram_tensor("w1c", [128, 6 * 512], F32R, kind="ExternalInput")
    wr_d = nc.dram_tensor("wrc", [128, 4 * 128], F32R, kind="ExternalInput")
    wl_d = nc.dram_tensor("wlc", [128, 9 * 128], F32R, kind="ExternalInput")
    wg_d = nc.dram_tensor("wgc", [128, 9 * 128], F32R, kind="ExternalInput")
    wro_d = nc.dram_tensor("wroc", [128, 2 * 512], F32R, kind="ExternalInput")
    wo_d = nc.dram_tensor("woc", [128, 4 * 512], F32R, kind="ExternalInput")
    qt_d = nc.dram_tensor("qtc", [128, 4 * 640], F32R, kind="ExternalInput")
    b1_d = nc.dram_tensor("b1c", [128, 4], F32, kind="ExternalInput")
    b2_d = nc.dram_tensor("b2c", [128, 4], F32, kind="ExternalInput")
    id_d = nc.dram_tensor("idc", [128, 128], F32, kind="ExternalInput")

    feats_o = nc.dram_tensor("feats_o", [spc, N, D], F32, kind="ExternalOutput")
    ksum_o = nc.dram_tensor("ksum_o", [spc, C, N], F32, kind="ExternalOutput")

    with tile.TileContext(nc) as tc:
        with (
            tc.tile_pool(name="consts", bufs=1) as consts,
            tc.tile_pool(name="io", bufs=2) as io,
            tc.tile_pool(name="acts", bufs=1) as acts,
            tc.tile_pool(name="tmps", bufs=2) as tmps,
            tc.tile_pool(name="ps", bufs=2, space="PSUM") as ps,
        ):
            w1 = consts.tile([128, 6 * 512], F32R, name="w1", tag="w1")
            nc.sync.dma_start(out=w1, in_=w1_d[:, :])
            wr = consts.tile([128, 4 * 128], F32R, name="wr", tag="wr")
            nc.sync.dma_start(out=wr, in_=wr_d[:, :])
            wl = consts.tile([128, 9 * 128], F32R, name="wl", tag="wl")
            nc.sync.dma_start(out=wl, in_=wl_d[:, :])
            wg = consts.tile([128, 9 * 128], F32R, name="wg", tag="wg")
            nc.sync.dma_start(out=wg, in_=wg_d[:, :])
            wro = consts.tile([128, 2 * 512], F32R, name="wro", tag="wro")
            nc.sync.dma_start(out=wro, in_=wro_d[:, :])
            wo = consts.tile([128, 4 * 512], F32R, name="wo", tag="wo")
            nc.sync.dma_start(out=wo, in_=wo_d[:, :])
            qt = consts.tile([128, 4 * 640], F32R, name="qt", tag="qt")
            nc.sync.dma_start(out=qt, in_=qt_d[:, :])
            b1 = consts.tile([128, 4], F32, name="b1", tag="b1")
            nc.sync.dma_start(out=b1, in_=b1_d[:, :])
            b2 = consts.tile([128, 4], F32, name="b2", tag="b2")
            nc.sync.dma_start(out=b2, in_=b2_d[:, :])
            ident = consts.tile([128, 128], F32, name="ident", tag="ident")
            nc.sync.dma_start(out=ident, in_=id_d[:, :])

            for s in range(spc):
                # ---- load tokens (transposed on host to [DIN, N]) ----
                tok = io.tile([128, 6, N], F32R, name=f"tok{s}", tag="tok")
                nc.sync.dma_start(
                    out=tok, in_=tokt_d[s].rearrange("(k p) n -> p k n", p=128)
                )

                # ---- stage A: adapter 1x1 + BN1 + ReLU ----
                y1 = acts.tile([128, 4, N], F32R, name=f"y1_{s}", tag="y1")
                for dc in range(4):
                    for nf in range(2):
                        pa = ps.tile([128, 512], F32, name=f"pa{s}_{dc}_{nf}",
                                     tag="mm", bufs=2)
                        for k in range(6):
                            o = k * 512 + dc * 128
                            nc.tensor.matmul(
                                pa,
                                lhsT=_r(w1[:, o:o + 128]),
                                rhs=_r(tok[:, k, nf * 512:(nf + 1) * 512]),
                                start=(k == 0), stop=(k == 5),
                            )
                        nc.scalar.activation(
                            y1[:, dc, nf * 512:(nf + 1) * 512], pa, AF.Relu,
                            bias=b1[:, dc:dc + 1],
                        )

                # ---- stage B: reduce 1x1 + ReLU into padded layout ----
                zp = acts.tile([128, 2 * MARGIN + W * W], F32R, name=f"zp{s}",
                               tag="zp")
                nc.gpsimd.memset(zp.bitcast(F32), 0.0)
                # valid region: rows 2..34, cols 2..34 of the 36x36 pad space
                zpv = zp[:, MARGIN + 2 * W + 2: MARGIN + 2 * W + 2 + 32 * W]
                zpv = zpv.rearrange("p (y x) -> p y x", x=W)
                for nf in range(2):
                    pb = ps.tile([128, 512], F32, name=f"pb{s}_{nf}", tag="mm",
                                 bufs=2)
                    for k in range(4):
                        nc.tensor.matmul(
                            pb,
                            lhsT=_r(wr[:, k * 128:(k + 1) * 128]),
                            rhs=_r(y1[:, k, nf * 512:(nf + 1) * 512]),
                            start=(k == 0), stop=(k == 3),
                        )
                    nc.scalar.activation(
                        zpv[:, nf * 16:(nf + 1) * 16, 0:32],
                        pb.rearrange("p (a b) -> p a b", b=32), AF.Relu,
                    )

                # ---- stage C: two 3x3 convs (dilation 1 and 2) ----
                branches = []
                for wt, dil, zname in ((wl, 1, "zl"), (wg, 2, "zg")):
                    zbr = acts.tile([128, N], F32R, name=f"{zname}_{s}", tag=zname)
                    branches.append(zbr)
                    for f in range(3):
                        pc = ps.tile([128, 432], F32, name=f"pc{zname}{s}_{f}",
                                     tag="pc", bufs=2)
                        for t in range(9):
                            dy, dx = t // 3 - 1, t % 3 - 1
                            off = MARGIN + f * 432 + (dy * W + dx) * dil
                            nc.tensor.matmul(
                                pc,
                                lhsT=_r(wt[:, t * 128:(t + 1) * 128]),
                                rhs=_r(zp[:, off:off + 432]),
                                start=(t == 0), stop=(t == 8),
                            )
                        # rows of pad space in this psum third: [12f, 12f+12)
                        r0, r1 = max(2, 12 * f), min(34, 12 * f + 12)
                        nr = r1 - r0
                        pcv = pc.rearrange("p (r c) -> p r c", c=W)
                        nc.scalar.activation(
                            zbr[:, (r0 - 2) * 32:(r1 - 2) * 32]
                            .rearrange("p (a b) -> p a b", b=32),
                            pcv[:, r0 - 12 * f:r1 - 12 * f, 2:34], AF.Relu,
                        )
                zl, zg = branches

                # ---- stage D: rcb_out 1x1 (BN2 folded) + residual + ReLU ----
                y2 = acts.tile([128, 4, N], F32R, name=f"y2_{s}", tag="y2")
                for dc in range(4):
                    for nf in range(2):
                        pd = ps.tile([128, 512], F32, name=f"pd{s}_{dc}_{nf}",
                                     tag="mm", bufs=2)
                        nc.tensor.matmul(
                            pd, lhsT=_r(wro[:, dc * 128:(dc + 1) * 128]),
                            rhs=_r(zl[:, nf * 512:(nf + 1) * 512]),
                            start=True, stop=False,
                        )
                        nc.tensor.matmul(
                            pd, lhsT=_r(wro[:, 512 + dc * 128:512 + (dc + 1) * 128]),
                            rhs=_r(zg[:, nf * 512:(nf + 1) * 512]),
                            start=False, stop=True,
                        )
                        dtmp = tmps.tile([128, 512], F32, name=f"dt{s}_{dc}_{nf}",
                                         tag="dtmp")
                        nc.vector.tensor_add(
                            dtmp, pd,
                            y1[:, dc, nf * 512:(nf + 1) * 512].bitcast(F32),
                        )
                        nc.scalar.activation(
                            y2[:, dc, nf * 512:(nf + 1) * 512], dtmp, AF.Relu,
                            bias=b2[:, dc:dc + 1],
                        )

                # ---- stage E: x3 = w_out @ y2, emitted in [n, d] layout ----
                x3 = acts.tile([128, 8, 512], F32, name=f"x3_{s}", tag="x3")
                ssq = tmps.tile([128, 8], F32, name=f"ssq{s}", tag="ssq")
                for ch in range(8):
                    n0 = ch * 128
                    pe_ = ps.tile([128, 512], F32, name=f"pe{s}_{ch}", tag="mm",
                                  bufs=2)
                    for k in range(4):
                        nc.tensor.matmul(
                            pe_, lhsT=_r(y2[:, k, n0:n0 + 128]),
                            rhs=_r(wo[:, k * 512:(k + 1) * 512]),
                            start=(k == 0), stop=(k == 3),
                        )
                    nc.scalar.copy(x3[:, ch, :], pe_)
                    sq = tmps.tile([128, 512], F32, name=f"sq{s}_{ch}", tag="sq")
                    nc.vector.scalar_tensor_tensor(
                        out=sq, in0=x3[:, ch, :], scalar=1.0, in1=x3[:, ch, :],
                        op0=ALU.mult, op1=ALU.mult,
                        accum_out=ssq[:, ch:ch + 1],
                    )

                # ---- rsqrt batch: rinv4 = 4/sqrt(ssq) (DVE only) ----
                useed = tmps.tile([128, 8], I32, name=f"useed{s}", tag="useed")
                nc.vector.memset(useed, 0x5F3759DF)
                u = tmps.tile([128, 8], I32, name=f"u{s}", tag="u")
                nc.vector.tensor_scalar(
                    out=u, in0=ssq.bitcast(I32), scalar1=1, scalar2=None,
                    op0=ALU.logical_shift_right,
                )
                nc.vector.tensor_sub(u, useed, u)
                y_ = tmps.tile([128, 8], F32, name=f"yr{s}", tag="yr")
                a_ = tmps.tile([128, 8], F32, name=f"ar{s}", tag="ar")
                nc.vector.tensor_mul(a_, u.bitcast(F32), u.bitcast(F32))
                nc.vector.tensor_mul(a_, a_, ssq)
                nc.vector.tensor_scalar(out=a_, in0=a_, scalar1=-0.5, scalar2=1.5,
                                        op0=ALU.mult, op1=ALU.add)
                nc.vector.tensor_mul(y_, u.bitcast(F32), a_)
                for _ in range(2):
                    nc.vector.tensor_mul(a_, y_, y_)
                    nc.vector.tensor_mul(a_, a_, ssq)
                    nc.vector.tensor_scalar(out=a_, in0=a_, scalar1=-0.5,
                                            scalar2=1.5, op0=ALU.mult, op1=ALU.add)
                    nc.vector.tensor_mul(y_, y_, a_)
                rinv4 = tmps.tile([128, 8], F32, name=f"rinv4_{s}", tag="rinv4")
                nc.vector.tensor_scalar(out=rinv4, in0=y_, scalar1=4.0,
                                        scalar2=None, op0=ALU.mult)

                # ---- feats out: x3 * (1/||x3||) ----
                for ch in range(8):
                    ft = tmps.tile([128, 512], F32, name=f"ft{s}_{ch}", tag="ft",
                                   bufs=3)
                    nc.vector.tensor_scalar(
                        out=ft, in0=x3[:, ch, :], scalar1=rinv4[:, ch:ch + 1],
                        scalar2=0.25, op0=ALU.mult, op1=ALU.mult,
                    )
                    nc.sync.dma_start(
                        out=feats_o[s, ch * 128:(ch + 1) * 128, :], in_=ft
                    )

                # ---- stage F: sims via Q = protos @ w_out, exp, per-class sum
                kst = tmps.tile([C, N], F32, name=f"kst{s}", tag="kst")
                for ch in range(8):
                    n0 = ch * 128
                    ps0 = ps.tile([128, 320], F32, name=f"ps0_{s}_{ch}", tag="s",
                                  bufs=3)
                    ps1 = ps.tile([128, 320], F32, name=f"ps1_{s}_{ch}", tag="s",
                                  bufs=3)
                    for k in range(4):
                        nc.tensor.matmul(
                            ps0, lhsT=_r(y2[:, k, n0:n0 + 128]),
                            rhs=_r(qt[:, k * 640:k * 640 + 320]),
                            start=(k == 0), stop=(k == 3),
                        )
                    for k in range(4):
                        nc.tensor.matmul(
                            ps1, lhsT=_r(y2[:, k, n0:n0 + 128]),
                            rhs=_r(qt[:, k * 640 + 320:(k + 1) * 640]),
                            start=(k == 0), stop=(k == 3),
                        )
                    es = tmps.tile([128, 640], F32, name=f"es{s}_{ch}", tag="es")
                    nc.scalar.activation(es[:, 0:320], ps0, AF.Exp,
                                         scale=rinv4[:, ch:ch + 1])
                    nc.scalar.activation(es[:, 320:640], ps1, AF.Exp,
                                         scale=rinv4[:, ch:ch + 1])
                    ks = tmps.tile([128, C], F32, name=f"ks{s}_{ch}", tag="ks")
                    nc.vector.reduce_sum(
                        ks, es.rearrange("p (c k) -> p c k", k=K), axis=AX.X
                    )
                    pt = ps.tile([C, 128], F32, name=f"pt{s}_{ch}", tag="pt",
                                 bufs=1)
                    nc.tensor.transpose(pt, ks, ident)
                    nc.scalar.copy(kst[:, n0:n0 + 128], pt)
                nc.sync.dma_start(out=ksum_o[s], in_=kst)

    nc.finalize()
    return nc


_NC_CACHE = {}


def _get_nc(spc=SPC):
    if spc not in _NC_CACHE:
        _NC_CACHE[spc] = build_bass(spc)
    return _NC_CACHE[spc]


def prep_weights(w_in, bn1_gamma, bn1_beta, w_reduce, w_local, w_global,
                 w_rcb_out, bn2_gamma, bn2_beta, w_out, protos):
    """Host-side packing of all weights into device layouts."""
    f32 = np.float32
    sc1 = (np.asarray(bn1_gamma, f32) / np.sqrt(np.float32(1.0 + BN_EPS)))
    sc2 = (np.asarray(bn2_gamma, f32) / np.sqrt(np.float32(1.0 + BN_EPS)))
    w1f = np.asarray(w_in, f32) * sc1[:, None]           # [512, 768]
    wrof = np.asarray(w_rcb_out, f32) * sc2[:, None]     # [512, 256]

    def pack_T(w, kchunks, mcols):
        # w [out, in] -> lhsT layout [128, kchunks*mcols]:
        # arr[p, k*mcols + o] = w[o, k*128 + p]
        wt = np.ascontiguousarray(w.T)                   # [in, out]
        return np.ascontiguousarray(
            wt.reshape(kchunks, 128, mcols).transpose(1, 0, 2)
        ).reshape(128, kchunks * mcols)

    w1c = pack_T(w1f, 6, 512)
    wrc = pack_T(np.asarray(w_reduce, f32), 4, 128)
    wroc = pack_T(wrof, 2, 512)
    woc = pack_T(np.asarray(w_out, f32), 4, 512)

    q = (np.asarray(protos, np.float64).reshape(C * K, D)
         @ np.asarray(w_out, np.float64))                # [640, 512]
    qtc = pack_T(q.astype(f32), 4, 640)

    def pack_conv(w):
        # w [o, i, ky, kx] -> [128, 9*128]: arr[i, t*128 + o]
        return np.ascontiguousarray(
            np.asarray(w, f32).transpose(1, 2, 3, 0)
        ).reshape(128, 9 * 128)

    wlc = pack_conv(w_local)
    wgc = pack_conv(w_global)

    b1c = np.ascontiguousarray(np.asarray(bn1_beta, f32).reshape(4, 128).T)
    b2c = np.ascontiguousarray(np.asarray(bn2_beta, f32).reshape(4, 128).T)
    idc = np.eye(128, dtype=f32)
    return dict(w1c=w1c, wrc=wrc, wlc=wlc, wgc=wgc, wroc=wroc, woc=woc,
                qtc=qtc, b1c=b1c, b2c=b2c, idc=idc)


def make_in_map(tokens_shard, weights):
    tokt = np.ascontiguousarray(
        np.asarray(tokens_shard, np.float32).transpose(0, 2, 1)
    )
    m = {"tokt": tokt}
    m.update(weights)
    return m


def postprocess(feats_list, ksum_list, logit_scale):
    feats = np.concatenate(feats_list, axis=0)            # [B, N, D]
    ksumt = np.concatenate(ksum_list, axis=0)             # [B, C, N]
    nb = feats.shape[0]
    act_maps = (TAU * (np.log(ksumt) - np.log(np.float32(K)))).astype(np.float32)
    act_maps = act_maps.reshape(nb, C, S, S)
    nsum = ksumt.astype(np.float64).sum(axis=-1)          # [B, C]
    ls = float(np.asarray(logit_scale).reshape(-1)[0])
    logits = (TAU * ls * (np.log(nsum) - np.log(float(K)) - np.log(float(N))))
    return logits.astype(np.float32), act_maps, feats


def kernel(tokens, w_in, bn1_gamma, bn1_beta, w_reduce, w_local, w_global,
           w_rcb_out, bn2_gamma, bn2_beta, w_out, protos, logit_scale):
    global LAST_RESULT
    weights = prep_weights(w_in, bn1_gamma, bn1_beta, w_reduce, w_local,
                           w_global, w_rcb_out, bn2_gamma, bn2_beta, w_out,
                           protos)
    tokens = np.asarray(tokens, np.float32)
    in_maps = [
        make_in_map(tokens[c * SPC:(c + 1) * SPC], weights)
        for c in range(N_CORES)
    ]
    nc = _get_nc(SPC)
    res = run_bass_kernel_spmd(nc, in_maps, core_ids=list(range(N_CORES)),
                               trace=TRACE)
    LAST_RESULT = res
    feats_list = [r["feats_o"] for r in res.results]
    ksum_list = [r["ksum_o"] for r in res.results]
    return postprocess(feats_list, ksum_list, logit_scale)
